# revision 23
# baseline (speedup 1.0000x reference)
"""Trainium2 Bass kernel for nn_Attention (dense transformer block).

Reference computation (per batch element b, n = 32*32 = 1024 tokens, c = 512,
8 heads x 64 dim):
    qkv  = x @ w_qkv                      # [n, 3c]
    q,k,v per head; dots = q k^T / sqrt(d); attn = softmax(dots, axis=-1)
    out  = attn @ v  -> concat heads -> @ w_out + b_out

Sharding: data-parallel over the batch (8 cores x 1 batch element each),
weights replicated. No collectives needed.

Per-core dataflow (all layouts chosen so no engine ever needs to move data
across partitions):
  - x [n, c] is loaded and PE-transposed to xT [c, n].
  - qkT[f, t] (f = q/k feature) computed directly as w_qkv^T x^T using
    w_qkv as lhsT (natural layout) -> q^T / k^T per head fall out as
    64-partition slices.
  - v computed in natural layout [t, f] and stored with a ones column
    appended per head (vx[.., 65]); the attn@v matmul with lhsT = [v | 1]
    then yields outT rows 0..63 = (attn @ v)^T and row 64 = softmax sums.
  - dots are computed TRANSPOSED (dotsT = k^T^T q^T, i.e. lhsT=kT, rhs=qT),
    softmax exp runs on ACT directly PSUM->SBUF (scale fused), and the
    unnormalized attn@v accumulates over j-chunks.
  - normalization: 1/sums via DVE reciprocal_approx_fast on the sums row
    (lane 64), gpsimd partition_broadcast to 64 partitions, one fused
    DVE multiply that also evacuates PSUM -> outcatT.
  - outcatT [64, head, t] is exactly the lhsT needed for the output
    projection (K=64 chunks); + bias; DMA out.
"""

import os

import numpy as np

import concourse.bass as bass
import concourse.mybir as mybir
import concourse.tile as tile
from concourse import bacc
from concourse.bass_utils import run_bass_kernel_spmd
from concourse.masks import make_identity

N_CORES = 8
B, HH, WW, C = 8, 32, 32, 512
N = HH * WW          # 1024 tokens
HEADS, D = 8, 64     # head dim
F32 = mybir.dt.float32
NT = N // 128        # 8 token tiles
CC = C // 128        # 4 contraction chunks of 128
SCALE = float(D) ** -0.5

# matmul compute dtype: float32r = single-pass fp32 matmul (fast, slightly
# reduced multiply precision), float32 = exact but 4x slower on PE.
# Per-stage matmul dtypes. fp16 (10-bit mantissa, 2-byte PE streaming = full
# rate) for the qkv/dots side and the attention-value side costs ~5e-4 rel
# error total; the output projection stays float32r (single-pass fp32 matmul,
# tf32-like multiply) to protect the final absmax. 4-byte moving operands
# stream at half rate, which is why fp16 wins ~1.4x end-to-end.
MM_DT = getattr(mybir.dt, os.environ.get("ATTN_MM_DT", "float32r"))
_e = os.environ.get
DT_X = getattr(mybir.dt, _e("ATTN_DT_X")) if _e("ATTN_DT_X") else mybir.dt.float16
DT_ATT = getattr(mybir.dt, _e("ATTN_DT_ATT")) if _e("ATTN_DT_ATT") else mybir.dt.float16
DT_OUT = getattr(mybir.dt, _e("ATTN_DT_OUT")) if _e("ATTN_DT_OUT") else None
if _e("ATTN_DT_X") == "none":
    DT_X = None
if _e("ATTN_DT_ATT") == "none":
    DT_ATT = None


def _emit(tc, x, w_qkv, w_out, b_out, out, loop_iters=None):
    nc = tc.nc
    Exp = mybir.ActivationFunctionType.Exp

    def mm(o, lhsT, rhs, **kw):
        nc.tensor.matmul(o, lhsT=lhsT, rhs=rhs, **kw)

    with (
        tc.tile_pool(name="const", bufs=1) as const,
        tc.tile_pool(name="xp", bufs=2) as xp,
        tc.tile_pool(name="pTp", bufs=4) as pTp,
        tc.tile_pool(name="rsp", bufs=2) as rsp,
        tc.tile_pool(name="rbp", bufs=2) as rbp,
        tc.tile_pool(name="yp", bufs=2) as yp,
        tc.tile_pool(name="ps1", bufs=2, space="PSUM") as ps1,
        tc.tile_pool(name="psD", bufs=2, space="PSUM") as psD,
    ):
        if loop_iters is not None:
            with tc.For_i(0, loop_iters, 1) as _i:
                _emit_body(tc, x, w_qkv, w_out, b_out, out,
                           const, xp, pTp, rsp, rbp, yp, ps1, psD)
        else:
            _emit_body(tc, x, w_qkv, w_out, b_out, out,
                       const, xp, pTp, rsp, rbp, yp, ps1, psD)


PHASES = set(os.environ.get("ATTN_PHASES", "A,B,C,Dd,Da,Dn,E").split(","))


def _emit_body(tc, x, w_qkv, w_out, b_out, out,
               const, xp, pTp, rsp, rbp, yp, ps1, psD):
    nc = tc.nc
    Exp = mybir.ActivationFunctionType.Exp

    def mm(o, lhsT, rhs, **kw):
        nc.tensor.matmul(o, lhsT=lhsT, rhs=rhs, **kw)

    if True:
        ident = const.tile([128, 128], F32)
        make_identity(nc, ident)
        identr = const.tile([128, 128], DT_X or MM_DT)
        nc.vector.tensor_copy(identr, ident)

        wqkv_sb = const.tile([128, CC, 3 * C], DT_X or MM_DT)
        wout_sb = const.tile([64, HEADS, C], DT_OUT or MM_DT)
        if mybir.dt.size(DT_X or MM_DT) == 4:
            nc.sync.dma_start(out=wqkv_sb, in_=w_qkv.rearrange("(cc p) f -> p cc f", p=128).bitcast(DT_X or MM_DT))
        else:
            wq_st = const.tile([128, CC, 3 * C], F32)
            nc.sync.dma_start(out=wq_st, in_=w_qkv.rearrange("(cc p) f -> p cc f", p=128))
            nc.vector.tensor_copy(wqkv_sb, wq_st)
        if mybir.dt.size(DT_OUT or MM_DT) == 4:
            nc.sync.dma_start(out=wout_sb, in_=w_out.rearrange("(h p) f -> p h f", p=64).bitcast(DT_OUT or MM_DT))
        else:
            wo_st = const.tile([64, HEADS, C], F32)
            nc.sync.dma_start(out=wo_st, in_=w_out.rearrange("(h p) f -> p h f", p=64))
            nc.vector.tensor_copy(wout_sb, wo_st)
        bias_sb = const.tile([128, C], F32)
        bias_bcast = bass.AP(tensor=b_out.tensor, offset=b_out.offset,
                             ap=[[0, 128]] + list(b_out.ap))
        nc.sync.dma_start(out=bias_sb, in_=bias_bcast)

        xT = const.tile([128, CC, N], DT_X or MM_DT)       # xT[p, cc, t] = x[t, cc*128+p]
        qkT = const.tile([128, 2 * CC, N], DT_X or MM_DT)  # qkT[p, ft, t] = (x w_qk)^T
        vx = const.tile([128, NT, HEADS, D + 1], DT_ATT or MM_DT)  # v + ones column
        outcatT = const.tile([65, HEADS, N], DT_OUT or MM_DT)

        ones_sb = const.tile([128, 1], F32)
        nc.vector.memset(ones_sb, 1.0)
        nc.vector.tensor_copy(vx[:, :, :, D:D + 1],
                              ones_sb[:, 0:1].to_broadcast([128, NT, HEADS, 1]))

        # ---- load + transpose x -> xT ----
        for tt in range(NT if "A" in PHASES else 0):
            xl = xp.tile([128, C], DT_X or MM_DT, tag="xl")
            if mybir.dt.size(DT_X or MM_DT) == 4:
                nc.sync.dma_start(out=xl, in_=x[tt * 128:(tt + 1) * 128, :].bitcast(DT_X or MM_DT))
            else:
                xl_st = xp.tile([128, C], F32, tag="xst")
                nc.sync.dma_start(out=xl_st, in_=x[tt * 128:(tt + 1) * 128, :])
                nc.vector.tensor_copy(xl, xl_st)
            tp = ps1.tile([128, 512], DT_X or MM_DT, tag="ps")
            for cc in range(CC):
                nc.tensor.transpose(tp[:, cc * 128:(cc + 1) * 128],
                                    xl[:, cc * 128:(cc + 1) * 128], identr)
            nc.vector.tensor_copy(xT[:, :, tt * 128:(tt + 1) * 128],
                                  tp.rearrange("p (cc t) -> p cc t", cc=CC))

        # ---- qkT = (w_qk)^T x^T ----
        for ft in range(2 * CC if "B" in PHASES else 0):
            qk0 = ps1.tile([128, 512], F32, tag="ps")
            qk1 = ps1.tile([128, 512], F32, tag="ps")
            for cc in range(CC):
                for half, qk in ((0, qk0), (1, qk1)):
                    mm(qk, wqkv_sb[:, cc, ft * 128:(ft + 1) * 128],
                       xT[:, cc, half * 512:(half + 1) * 512],
                       start=(cc == 0), stop=(cc == CC - 1))
            nc.vector.tensor_copy(qkT[:, ft, 0:512], qk0)
            nc.vector.tensor_copy(qkT[:, ft, 512:1024], qk1)

        # ---- v = x w_v (natural layout, strided into vx) ----
        for tt in range(NT if "C" in PHASES else 0):
            vps = ps1.tile([128, 512], F32, tag="ps")
            for cc in range(CC):
                mm(vps, xT[:, cc, tt * 128:(tt + 1) * 128],
                   wqkv_sb[:, cc, 2 * C:3 * C],
                   start=(cc == 0), stop=(cc == CC - 1))
            nc.vector.tensor_copy(vx[:, tt, :, 0:D],
                                  vps.rearrange("p (h d) -> p h d", h=HEADS))

        # ---- attention, head pairs packed into PE row groups ----
        # heads (2g, 2g+1) live at qkT partitions 0..63 / 64..127; their two
        # K=64 dots matmuls occupy different PE row groups and run
        # concurrently, writing the two halves (banks) of one dp tile.
        # dots for the pair stream through [128, 1536] dp tiles (3 blocks of
        # 512) so each ACT exp op covers 3 blocks -- fewer, larger ACT ops.
        DPB = 3
        for g in range(HEADS // 2 if "Dd" in PHASES else 0):
            for ihalf in range(2):
                isl = slice(ihalf * 512, (ihalf + 1) * 512)
                o_lo = ps1.tile([65, 512], F32, tag="ps")
                o_hi = ps1.tile([65, 512], F32, tag="ps")
                blocks = [(jc, hh) for jc in range(NT) for hh in (0, 1)]
                dp = None
                pend = []
                for b, (jc, hh) in enumerate(blocks):
                    pos = b % DPB
                    if pos == 0:
                        nblk = min(DPB, len(blocks) - b)
                        dp = psD.tile([128, nblk * 512], F32, tag="d")
                    jsl = slice(jc * 128, (jc + 1) * 128)
                    hp = hh * 64
                    mm(dp[:, pos * 512:(pos + 1) * 512],
                       qkT[hp:hp + 64, CC + g, jsl], qkT[hp:hp + 64, g, isl],
                       start=True, stop=True)
                    pend.append((jc, hh, pos))
                    if pos == nblk - 1:
                        pt = pTp.tile([128, nblk * 512], DT_ATT or MM_DT, tag="pt")
                        nc.scalar.activation(pt, dp, Exp, scale=SCALE)
                        if "Da" in PHASES:
                            for (pjc, phh, ppos) in pend:
                                o = o_hi if phh else o_lo
                                mm(o, vx[:, pjc, 2 * g + phh, :],
                                   pt[:, ppos * 512:(ppos + 1) * 512],
                                   start=(pjc == 0), stop=(pjc == NT - 1))
                        pend = []
                if "Da" in PHASES:
                    # evacuate fast: rows 0..63 = out^T (unnormalized), row 64
                    # = softmax sums; both land in outcatT (row 64 is spare).
                    nc.vector.tensor_copy(outcatT[:, 2 * g, isl], o_lo)
                    nc.vector.tensor_copy(outcatT[:, 2 * g + 1, isl], o_hi)
        # deferred normalization: per head, shift the sums row to partition 0
        # via a tiny DMA (recip_approx_fast / partition_broadcast only work at
        # partition 0 on HW), then scale in place.
        for h in range(HEADS if "Dn" in PHASES else 0):
            s0 = rsp.tile([1, N], DT_OUT or MM_DT, tag="s0")
            nc.sync.dma_start(out=s0, in_=outcatT[64:65, h, :])
            if mybir.dt.size(DT_OUT or MM_DT) != 4:
                s0f = rsp.tile([1, N], F32, tag="s0f")
                nc.vector.tensor_copy(s0f, s0)
                s0 = s0f
            else:
                s0 = s0.bitcast(F32)
            rs = rsp.tile([1, N], F32, tag="rs")
            nc.vector.reciprocal_approx_fast(rs, s0)
            rb = rbp.tile([64, N], F32, tag="rb")
            nc.gpsimd.partition_broadcast(rb, rs)
            nc.vector.tensor_mul(outcatT[0:64, h, :], outcatT[0:64, h, :], rb)

        # ---- output projection + bias ----
        for tt in range(NT if "E" in PHASES else 0):
            yps = ps1.tile([128, 512], F32, tag="ps")
            for h in range(HEADS):
                mm(yps, outcatT[0:64, h, tt * 128:(tt + 1) * 128],
                   wout_sb[:, h, :],
                   start=(h == 0), stop=(h == HEADS - 1))
            ysb = yp.tile([128, C], F32, tag="y")
            nc.vector.tensor_add(ysb, yps, bias_sb)
            nc.sync.dma_start(out=out[tt * 128:(tt + 1) * 128, :], in_=ysb)


def build_nc(loop_iters=None):
    nc = bacc.Bacc("TRN2", target_bir_lowering=False, debug=False)
    x = nc.declare_dram_parameter("x", [N, C], F32, isOutput=False).ap()
    w_qkv = nc.declare_dram_parameter("w_qkv", [C, 3 * C], F32, isOutput=False).ap()
    w_out = nc.declare_dram_parameter("w_out", [C, C], F32, isOutput=False).ap()
    b_out = nc.declare_dram_parameter("b_out", [C], F32, isOutput=False).ap()
    out = nc.declare_dram_parameter("out", [N, C], F32, isOutput=True).ap()
    with tile.TileContext(nc) as tc:
        _emit(tc, x, w_qkv, w_out, b_out, out, loop_iters=loop_iters)
    nc.compile()
    return nc


_NC_CACHE = {}


def _get_nc():
    key = str(MM_DT)
    if key not in _NC_CACHE:
        _NC_CACHE[key] = build_nc()
    return _NC_CACHE[key]


def run(inputs, trace=False):
    """Run on 8 NeuronCores; returns (full output, BassKernelResults)."""
    x = np.ascontiguousarray(np.asarray(inputs["x"], dtype=np.float32))
    w_qkv = np.ascontiguousarray(np.asarray(inputs["w_qkv"], dtype=np.float32))
    w_out = np.ascontiguousarray(np.asarray(inputs["w_out"], dtype=np.float32))
    b_out = np.ascontiguousarray(np.asarray(inputs["b_out"], dtype=np.float32))
    nc = _get_nc()
    in_maps = [
        {"x": x[i].reshape(N, C), "w_qkv": w_qkv, "w_out": w_out, "b_out": b_out}
        for i in range(N_CORES)
    ]
    res = run_bass_kernel_spmd(nc, in_maps, list(range(N_CORES)), trace=trace)
    full = np.stack([res.results[i]["out"] for i in range(N_CORES)])
    return full.reshape(B, HH, WW, C), res


def kernel(x, w_qkv, w_out, b_out):
    full, _ = run({"x": x, "w_qkv": w_qkv, "w_out": w_out, "b_out": b_out})
    return full
